# revision 18
# baseline (speedup 1.0000x reference)
"""MHA (B=2, S=2048, D=1024, H=16) on 8 Trainium2 NeuronCores - bf16.

Sharding: core c = (batch b = c//4, head-group g = c%4, 4 heads each).

v3 structural changes over the 223us baseline (cost model: matmul time =
moving-free-size cycles, independent of contraction/partition count):
 - AV matmul flipped: exp-scores tile [128k,128q] is the STATIONARY
   operand, V(+ones) [128k,65] the moving one: AV drops from 32768 to
   16640 PE cycles per head. Output lands as [q, 64hd+denom] in PSUM,
   is normalized by the per-partition reciprocal of the ones column
   (tensor_scalar) and PE-transposed (identity matmuls) back to [hd, q].
 - softmax exp on THREE engines (ACT exact / DVE+Pool schraudolph),
   strict A,D,A,P pattern; scores PSUM 4 banks deep and AV trailing 5
   key-blocks so the PE never waits on the exp round-trip.
 - per-head drain (transpose/copy/A2A-dma/collective) is emitted INSIDE
   the next head's kb loop so neither the PE FIFO nor Pool.SEQ stalls at
   head boundaries.
 - four per-head AllToAlls (15us fixed cost each, serialized on the
   collective cores): c0 starts one head earlier than the baseline's
   merged head-pair collective and c3 lands ~6us sooner.
 - out-projection in two full-K waves (even k-tiles, then odd).
 - weight DMAs merged (one copy per W) on the ACT DGE queue; x tiles
   keep the whole SP queue so the PE starts earlier and never starves.
"""

import numpy as np
import ml_dtypes

B, S, D, H = 2, 2048, 1024, 16
HD = D // H          # 64
GH = 4               # heads per core
CORES = 8
QS = S // CORES      # 256 per-core final sequence slice (per batch)
P = 128
KT = D // P          # 8 k-tiles of the model dim
VW = HD + 1          # 65: per-head V width incl. ones column

A16 = 128.0 / float(np.log(2.0))   # schraudolph slope (bf16-bit domain)
B16 = 16255.35                     # schraudolph offset (tuned)
LNC = 0.03638                      # ln(c): ACT path matches schraudolph scale

_CACHE = {}

# exp engine shares per head (64 tiles): ACT exact / DVE / Pool schraudolph
EXP_SHARES = {"A": 34, "D": 30}


def _exp_engine_pattern():
    shares = dict(EXP_SHARES)
    counts = {k: 0 for k in shares}
    seq = []
    for i in range(64):
        pick = max(shares, key=lambda k: shares[k] * (i + 1) / 64 - counts[k])
        counts[pick] += 1
        seq.append(pick)
    return seq


PAT = _exp_engine_pattern()


def _build_nc():
    import concourse.mybir as mybir
    import concourse.tile as tile
    from concourse import bacc
    from concourse import masks

    F32 = mybir.dt.float32
    BF16 = mybir.dt.bfloat16
    U16 = mybir.dt.uint16
    EXP = mybir.ActivationFunctionType.Exp
    IDN = mybir.ActivationFunctionType.Identity
    MUL = mybir.AluOpType.mult
    ADD = mybir.AluOpType.add

    nc = bacc.Bacc("TRN2", target_bir_lowering=False, debug=False,
                   num_devices=CORES)

    d_x = nc.dram_tensor("x16", [D, S], BF16, kind="ExternalInput")
    d_wq = nc.dram_tensor("wq16", [D, 256], BF16, kind="ExternalInput")
    d_wk = nc.dram_tensor("wk16", [D, 256], BF16, kind="ExternalInput")
    d_wv = nc.dram_tensor("wv16", [D, 256], BF16, kind="ExternalInput")
    d_wo = nc.dram_tensor("wo16", [D, D], BF16, kind="ExternalInput")
    d_bq = nc.dram_tensor("bq2", [P, 2], F32, kind="ExternalInput")
    d_bk = nc.dram_tensor("bk2", [P, 2], F32, kind="ExternalInput")
    d_vi = nc.dram_tensor("vib1", [1, 256], F32, kind="ExternalInput")
    d_bo = nc.dram_tensor("bo1", [1, D], F32, kind="ExternalInput")
    d_y = nc.dram_tensor("y", [B, QS, D], BF16, kind="ExternalOutput")

    # AV psum packing: chains 0-6 -> bankA, 7-13 -> bankB, 14-15 -> bankC
    AV_SPLIT = [(0, 7), (7, 7), (14, 2)]

    def av_loc(j):
        for t, (base, n) in enumerate(AV_SPLIT):
            if base <= j < base + n:
                return t, j - base
        raise AssertionError

    with tile.TileContext(nc) as tc:
        with (
            tc.tile_pool(name="statics", bufs=1) as st,
            tc.tile_pool(name="dram", bufs=1, space="DRAM") as dram,
        ):
            bq = st.tile([P, 2], F32, tag="bq", name="bq")
            bk = st.tile([P, 2], F32, tag="bk", name="bk")
            lnc = st.tile([P, 1], F32, tag="lnc", name="lnc")
            vib = st.tile([P, 256], F32, tag="vib", name="vib")
            bob = st.tile([P, D], F32, tag="bob", name="bob")
            ident = st.tile([P, P], BF16, tag="ident", name="ident")
            nc.vector.memset(lnc[:], LNC)
            masks.make_identity(nc, ident[:])

            # head pair tiles: partition = (h%2)*64 + hd  (PE base 0/64)
            qTp = [st.tile([P, S], BF16, tag=f"qT{m}", name=f"qT{m}")
                   for m in range(2)]
            kTp = [st.tile([P, S], BF16, tag=f"kT{m}", name=f"kT{m}")
                   for m in range(2)]
            vaug = [st.tile([P, GH * VW], BF16, tag=f"va{i}", name=f"va{i}")
                    for i in range(16)]
            pay = [st.tile([HD, S], BF16, tag=f"pay{h}", name=f"pay{h}")
                   for h in range(GH)]
            woT = st.tile([P, KT * D], BF16, tag="woT", name="woT")
            # merged A2A readback: aoE/aoO[:, (bb*4+gp)*256 + q] for even/odd
            # k-tiles (lower 64 partitions = first head of the pair)
            aoE = st.tile([P, 8 * QS], BF16, tag="aoE", name="aoE")
            aoO = st.tile([P, 8 * QS], BF16, tag="aoO", name="aoO")

            # ---- projections ----
            with (
                tc.tile_pool(name="proj", bufs=1) as pr,
                tc.tile_pool(name="pj", bufs=5, space="PSUM") as pj,
                tc.tile_pool(name="pv", bufs=2, space="PSUM") as pvp,
            ):
                wqT = pr.tile([P, KT * 256], BF16, tag="wqT", name="wqT")
                wkT = pr.tile([P, KT * 256], BF16, tag="wkT", name="wkT")
                wvT = pr.tile([P, KT * 256], BF16, tag="wvT", name="wvT")
                xT = [pr.tile([P, S], BF16, tag=f"x{k}", name=f"x{k}")
                      for k in range(KT)]
                vi1 = pr.tile([1, 256], F32, tag="vi1", name="vi1")
                bo1 = pr.tile([1, D], F32, tag="bo1", name="bo1")
                # x owns the SP queue; weights go via the ACT DGE queue in
                # halves interleaved behind x0 so the PE's first chain group
                # starts ~4us in and never starves on x[k] or w[k]
                wq3 = wqT.rearrange("p (k c) -> p k c", k=KT)
                wk3 = wkT.rearrange("p (k c) -> p k c", k=KT)
                dq3 = d_wq.rearrange("(k p) c -> p k c", k=KT)
                dk3 = d_wk.rearrange("(k p) c -> p k c", k=KT)
                nc.scalar.dma_start(wk3[:, 0:4], dk3[:, 0:4])
                nc.scalar.dma_start(wq3[:, 0:4], dq3[:, 0:4])
                for k in range(0, 3):
                    nc.sync.dma_start(xT[k][:], d_x[k * P:(k + 1) * P, :])
                nc.scalar.dma_start(wk3[:, 4:8], dk3[:, 4:8])
                nc.scalar.dma_start(wq3[:, 4:8], dq3[:, 4:8])
                for k in range(3, KT):
                    nc.sync.dma_start(xT[k][:], d_x[k * P:(k + 1) * P, :])
                nc.scalar.dma_start(bq[:], d_bq[:])
                nc.scalar.dma_start(bk[:], d_bk[:])
                for i in range(16):
                    ones = vaug[i].rearrange("p (h w) -> p h w", w=VW)
                    nc.gpsimd.memset(ones[:, :, HD:VW], 1.0)

                # Q (bias-copy on ACT) / K (on DVE); k-major within groups of
                # chains so the PE streams behind the x DMA; pair 0 first
                qk_chains = []
                for m in range(2):
                    for nb in range(4):
                        qk_chains.append(("K", m, nb))
                        qk_chains.append(("Q", m, nb))
                for g0 in range(0, 16, 4):
                    grp = qk_chains[g0:g0 + 4]
                    tiles = [pj.tile([P, 512], F32, tag="pj", name="pj")
                             for _ in grp]
                    for k in range(KT):
                        for (pk, m, nb), ps in zip(grp, tiles):
                            w = wkT if pk == "K" else wqT
                            ws = w[:, k * 256 + m * P: k * 256 + (m + 1) * P]
                            nc.tensor.matmul(
                                ps[:], ws,
                                xT[k][:, nb * 512:(nb + 1) * 512],
                                start=(k == 0), stop=(k == KT - 1))
                    for (pk, m, nb), ps in zip(grp, tiles):
                        if pk == "K":
                            nc.vector.tensor_scalar(
                                kTp[m][:, nb * 512:(nb + 1) * 512], ps[:],
                                bk[:, m:m + 1], None, ADD)
                        else:
                            nc.scalar.activation(
                                qTp[m][:, nb * 512:(nb + 1) * 512], ps[:],
                                IDN, bias=bq[:, m:m + 1], scale=1.0)

                # V weights + biases now; wo last (needed only at out-proj)
                nc.scalar.dma_start(
                    wvT.rearrange("p (k c) -> p k c", k=KT),
                    d_wv.rearrange("(k p) c -> p k c", k=KT))
                nc.scalar.dma_start(vi1[:], d_vi[:])
                nc.scalar.dma_start(bo1[:], d_bo[:])
                nc.gpsimd.partition_broadcast(vib[:], vi1[:])
                nc.gpsimd.partition_broadcast(bob[:], bo1[:])
                nc.scalar.dma_start(
                    woT.rearrange("p (k c) -> p k c", k=KT),
                    d_wo.rearrange("(k p) c -> p k c", k=KT))

                # V: natural layout [kpos, 4 heads x 64] + ones col
                for sb in range(16):
                    pv = pvp.tile([P, 256], F32, tag="pv", name="pv")
                    for k in range(KT):
                        nc.tensor.matmul(
                            pv[:], xT[k][:, sb * P:(sb + 1) * P],
                            wvT[:, k * 256:(k + 1) * 256],
                            start=(k == 0), stop=(k == KT - 1))
                    dst = vaug[sb].rearrange("p (h w) -> p h w", w=VW)
                    nc.vector.tensor_tensor(
                        dst[:, :, 0:HD],
                        pv.rearrange("p (h w) -> p h w", w=HD),
                        vib.rearrange("p (h w) -> p h w", w=HD), ADD)

            # ---- attention ----
            a_ins = [dram.tile([CORES * HD, QS], BF16, name=f"a_in{h}")
                     for h in range(GH)]
            a_outs = [dram.tile([CORES * HD, QS], BF16, name=f"a_out{h}")
                      for h in range(GH)]

            def readback(h):
                # h 0/1 -> aoE lower/upper half; 2/3 -> aoO
                dst = (aoE if h < 2 else aoO).rearrange(
                    "p (s q) -> p s q", s=8)[(h % 2) * HD:(h % 2 + 1) * HD]
                nc.sync.dma_start(
                    dst, a_outs[h].rearrange("(s p) q -> p s q", s=8))

            with (
                tc.tile_pool(name="exp", bufs=1) as exp_pool,
                tc.tile_pool(name="nrm", bufs=2) as nr,
                tc.tile_pool(name="psc", bufs=4, space="PSUM") as psc,
                tc.tile_pool(name="pav", bufs=1, space="PSUM") as pav,
                tc.tile_pool(name="ptp", bufs=1, space="PSUM") as ptp,
            ):
                LAG = 3

                def drain_head(h, attsb, avt):
                    """Normalize (recip of ones col) -> bf16, emitted at head
                    end on the vector engines; one broadcast multiply per
                    AV psum tile."""
                    rs = []
                    for t, (base, n) in enumerate(AV_SPLIT):
                        r = nr.tile([P, n], F32, tag=f"rs{t}", name=f"rs{t}")
                        den = avt[t].rearrange("p (c w) -> p c w", w=VW)
                        nc.vector.reciprocal(r[:], den[:, :, HD])
                        rs.append(r)
                    for j in range(16):
                        t, jj = av_loc(j)
                        nc.vector.tensor_scalar(
                            attsb[:, j * HD:(j + 1) * HD],
                            avt[t][:, jj * VW:jj * VW + HD],
                            rs[t][:, jj:jj + 1], None, MUL)

                def transpose_group(h, attsb, g):
                    """PE transpose of 8 chains + copy into pay + A2A dma of
                    the half."""
                    tp = ptp.tile([HD, 8 * P], BF16, tag="tp", name="tp")
                    for j8 in range(8):
                        j = g * 8 + j8
                        nc.tensor.matmul(
                            tp[:, j8 * P:(j8 + 1) * P],
                            attsb[:, j * HD:(j + 1) * HD], ident[:],
                            is_transpose=True, start=(j8 == 0),
                            stop=(j8 == 7), skip_group_check=True)
                    if g == 0:
                        nc.vector.tensor_copy(
                            pay[h][:, 0:1024], tp[:])
                    else:
                        nc.scalar.copy(pay[h][:, 1024:2048], tp[:])
                    dst = a_ins[h].rearrange("(j r) q -> r j q", j=CORES)
                    src = pay[h].rearrange("p (j q) -> p j q", j=CORES)
                    nc.sync.dma_start(dst[:, g * 4:(g + 1) * 4],
                                      src[:, g * 4:(g + 1) * 4])

                def issue_cc(h):
                    nc.gpsimd.collective_compute(
                        "AllToAll",
                        mybir.AluOpType.bypass,
                        replica_groups=[list(range(CORES))],
                        ins=[a_ins[h][:]],
                        outs=[a_outs[h][:]],
                    )

                prev = None  # deferred drain state of head h-1
                for h in range(GH):
                    ksl = kTp[h // 2][(h % 2) * HD:(h % 2 + 1) * HD]
                    qsl = qTp[h // 2][(h % 2) * HD:(h % 2 + 1) * HD]
                    # one tile per (kb, qt) exp instruction: single writer,
                    # so cross-engine exps of one key block never serialize
                    ex = [[exp_pool.tile([P, 512], BF16, tag=f"ex{i}_{q}",
                                         name=f"ex{i}_{q}") for q in range(4)]
                          for i in range(16)]
                    avt = [pav.tile([P, n * VW], F32, tag=f"av{t}",
                                    name=f"av{t}")
                           for t, (_, n) in enumerate(AV_SPLIT)]

                    def av_half(kb, half, ex=ex, avt=avt, h=h):
                        # 8 chains: stationary = exp tile q-slice (M=128),
                        # moving = V+ones (N=65); accumulate over key blocks
                        for j in range(half * 8, half * 8 + 8):
                            t, jj = av_loc(j)
                            nc.tensor.matmul(
                                avt[t][:, jj * VW:(jj + 1) * VW],
                                ex[kb][j // 4][:, (j % 4) * P:
                                               (j % 4 + 1) * P],
                                vaug[kb][:, h * VW:(h + 1) * VW],
                                start=(kb == 0 and jj == 0),
                                stop=(kb == 15 and jj == AV_SPLIT[t][1] - 1),
                                skip_group_check=True)

                    for kb in range(16):
                        for qt in range(4):
                            # trailing-AV halves interleaved before the score
                            # pairs for uniform PE production pacing
                            if kb >= LAG and qt == 0:
                                av_half(kb - LAG, 0)
                            elif kb >= LAG and qt == 2:
                                av_half(kb - LAG, 1)
                            sc = psc.tile([P, 512], F32, tag="sc", name="sc")
                            qo = qt * 512
                            nc.tensor.matmul(
                                sc[:], ksl[:, kb * P:(kb + 1) * P],
                                qsl[:, qo:qo + 512],
                                start=True, stop=True)
                            dst = ex[kb][qt][:]
                            pick = PAT[kb * 4 + qt]
                            if pick == "A":
                                nc.scalar.activation(
                                    dst, sc[:], EXP,
                                    bias=lnc[:, 0:1], scale=1.0)
                            else:
                                nc.vector.tensor_scalar(
                                    dst.bitcast(U16), sc[:],
                                    A16, B16, MUL, ADD)
                        # deferred drain of the previous head, spread so the
                        # PE/Pool FIFOs never block at the boundary
                        if prev is not None:
                            ph, pattsb = prev
                            if kb == 0:
                                transpose_group(ph, pattsb, 0)
                            elif kb == 1:
                                transpose_group(ph, pattsb, 1)
                            elif kb == 2:
                                issue_cc(ph)
                                if ph >= 1:
                                    readback(ph - 1)
                    for kb in range(16 - LAG, 16):
                        av_half(kb, 0)
                        av_half(kb, 1)

                    attsb = nr.tile([P, 16 * HD], BF16, tag="attsb",
                                    name="attsb")
                    drain_head(h, attsb, avt)
                    prev = (h, attsb)

                # tail: drain head 3 immediately
                transpose_group(3, prev[1], 0)
                transpose_group(3, prev[1], 1)
                issue_cc(3)
                readback(2)
                readback(3)

            # ---- out projection (my 256-row slice of each batch) ----
            with (
                tc.tile_pool(name="po", bufs=1, space="PSUM") as po,
                tc.tile_pool(name="yo", bufs=4) as yo,
            ):
                tiles = {}
                for key in [(bb, m, n) for bb in reversed(range(B))
                            for m in reversed(range(2))
                            for n in reversed(range(2))]:
                    tiles[key] = po.tile([P, 512], F32,
                                         tag="po{}{}{}".format(*key),
                                         name="po{}{}{}".format(*key))
                chains = [(bb, m, n, tiles[(bb, m, n)])
                          for bb in range(B) for m in range(2)
                          for n in range(2)]
                # three waves: even k-tiles full-K (aoE, after c0+c1), then
                # the head-2 and head-3 K=64 halves of the odd k-tiles
                # (after c2 / c3) so the PE keeps running while c3 lands
                for phase in range(3):
                    for bb, m, n, ps in chains:
                        for ki in range(4):
                            k = ki * 2 + (1 if phase > 0 else 0)
                            c0 = (bb * 4 + ki) * QS + m * P
                            if phase == 0:
                                src = aoE[:, c0:c0 + P]
                                wos = woT[:, k * D + n * 512:
                                          k * D + (n + 1) * 512]
                            else:
                                src = aoO[(phase - 1) * HD:phase * HD,
                                          c0:c0 + P]
                                wos = woT.rearrange(
                                    "p (k c) -> p k c", k=KT)[
                                    (phase - 1) * HD:phase * HD, k,
                                    n * 512:(n + 1) * 512]
                            nc.tensor.matmul(
                                ps[:], src, wos,
                                start=(phase == 0 and ki == 0),
                                stop=(phase == 2 and ki == 3))
                for ci, (bb, m, n, ps) in enumerate(chains):
                    ys = yo.tile([P, 512], BF16, tag="ys", name="ys")
                    nc.vector.tensor_tensor(
                        ys[:], ps[:], bob[:, n * 512:(n + 1) * 512], ADD)
                    nc.sync.dma_start(
                        d_y[bb, m * P:(m + 1) * P, n * 512:(n + 1) * 512],
                        ys[:])

    nc.compile()
    return nc


def get_nc():
    if "nc" not in _CACHE:
        _CACHE["nc"] = _build_nc()
    return _CACHE["nc"]


def make_in_maps(x, Wq, bq, Wk, bk, Wv, bv, Wo, bo):
    bf16 = ml_dtypes.bfloat16
    x = np.asarray(x, dtype=np.float32)
    Wq, Wk, Wv, Wo = (np.asarray(w, dtype=np.float32) for w in (Wq, Wk, Wv, Wo))
    bq, bk, bv, bo = (np.asarray(v, dtype=np.float32) for v in (bq, bk, bv, bo))
    scale = 1.0 / np.sqrt(np.float32(HD))

    wo16 = np.ascontiguousarray(Wo.T).astype(bf16)
    bo1 = bo.reshape(1, D)

    in_maps = []
    for cc in range(CORES):
        b, g = cc // 4, cc % 4
        sl = slice(g * 256, (g + 1) * 256)
        x16 = np.ascontiguousarray(x[b].T).astype(bf16)
        wq16 = np.ascontiguousarray((Wq[sl, :] * scale).T).astype(bf16)
        wk16 = np.ascontiguousarray(Wk[sl, :].T).astype(bf16)
        wv16 = np.ascontiguousarray(Wv[sl, :].T).astype(bf16)
        pp = np.arange(P)
        bq2 = np.stack([bq[g * 256 + m * P + pp] * scale for m in range(2)],
                       axis=1).astype(np.float32)
        bk2 = np.stack([bk[g * 256 + m * P + pp] for m in range(2)],
                       axis=1).astype(np.float32)
        vib1 = bv[sl].reshape(1, 256).astype(np.float32)
        in_maps.append({
            "x16": x16, "wq16": wq16, "wk16": wk16, "wv16": wv16,
            "wo16": wo16, "bq2": np.ascontiguousarray(bq2),
            "bk2": np.ascontiguousarray(bk2), "vib1": vib1, "bo1": bo1,
        })
    return in_maps


def assemble(results):
    out = np.empty((B, S, D), dtype=np.float32)
    for c in range(CORES):
        out[:, c * QS:(c + 1) * QS, :] = np.asarray(
            results[c]["y"], dtype=np.float32)
    return out


def kernel(**inputs):
    from concourse.bass_utils import run_bass_kernel_spmd

    nc = get_nc()
    in_maps = make_in_maps(**inputs)
    res = run_bass_kernel_spmd(nc, in_maps, list(range(CORES)), trace=False)
    return assemble(res.results)


# revision 19
# speedup vs baseline: 1.0048x; 1.0048x over previous
"""MHA (B=2, S=2048, D=1024, H=16) on 8 Trainium2 NeuronCores - bf16.

Sharding: core c = (batch b = c//4, head-group g = c%4, 4 heads each).

v3 structural changes over the 223us baseline (cost model: matmul time =
moving-free-size cycles, independent of contraction/partition count):
 - AV matmul flipped: exp-scores tile [128k,128q] is the STATIONARY
   operand, V(+ones) [128k,65] the moving one: AV drops from 32768 to
   16640 PE cycles per head. Output lands as [q, 64hd+denom] in PSUM,
   is normalized by the per-partition reciprocal of the ones column
   (tensor_scalar) and PE-transposed (identity matmuls) back to [hd, q].
 - softmax exp on THREE engines (ACT exact / DVE+Pool schraudolph),
   strict A,D,A,P pattern; scores PSUM 4 banks deep and AV trailing 5
   key-blocks so the PE never waits on the exp round-trip.
 - per-head drain (transpose/copy/A2A-dma/collective) is emitted INSIDE
   the next head's kb loop so neither the PE FIFO nor Pool.SEQ stalls at
   head boundaries.
 - four per-head AllToAlls (15us fixed cost each, serialized on the
   collective cores): c0 starts one head earlier than the baseline's
   merged head-pair collective and c3 lands ~6us sooner.
 - out-projection in two full-K waves (even k-tiles, then odd).
 - weight DMAs merged (one copy per W) on the ACT DGE queue; x tiles
   keep the whole SP queue so the PE starts earlier and never starves.
"""

import numpy as np
import ml_dtypes

B, S, D, H = 2, 2048, 1024, 16
HD = D // H          # 64
GH = 4               # heads per core
CORES = 8
QS = S // CORES      # 256 per-core final sequence slice (per batch)
P = 128
KT = D // P          # 8 k-tiles of the model dim
VW = HD + 1          # 65: per-head V width incl. ones column

A16 = 128.0 / float(np.log(2.0))   # schraudolph slope (bf16-bit domain)
B16 = 16255.35                     # schraudolph offset (tuned)
LNC = 0.03638                      # ln(c): ACT path matches schraudolph scale

_CACHE = {}

# exp engine shares per head (64 tiles): ACT exact / DVE / Pool schraudolph
EXP_SHARES = {"A": 33, "D": 31}


def _exp_engine_pattern():
    shares = dict(EXP_SHARES)
    counts = {k: 0 for k in shares}
    seq = []
    for i in range(64):
        pick = max(shares, key=lambda k: shares[k] * (i + 1) / 64 - counts[k])
        counts[pick] += 1
        seq.append(pick)
    return seq


PAT = _exp_engine_pattern()


def _build_nc():
    import concourse.mybir as mybir
    import concourse.tile as tile
    from concourse import bacc
    from concourse import masks

    F32 = mybir.dt.float32
    BF16 = mybir.dt.bfloat16
    U16 = mybir.dt.uint16
    EXP = mybir.ActivationFunctionType.Exp
    IDN = mybir.ActivationFunctionType.Identity
    MUL = mybir.AluOpType.mult
    ADD = mybir.AluOpType.add

    nc = bacc.Bacc("TRN2", target_bir_lowering=False, debug=False,
                   num_devices=CORES)

    d_x = nc.dram_tensor("x16", [D, S], BF16, kind="ExternalInput")
    d_wq = nc.dram_tensor("wq16", [D, 256], BF16, kind="ExternalInput")
    d_wk = nc.dram_tensor("wk16", [D, 256], BF16, kind="ExternalInput")
    d_wv = nc.dram_tensor("wv16", [D, 256], BF16, kind="ExternalInput")
    d_wo = nc.dram_tensor("wo16", [D, D], BF16, kind="ExternalInput")
    d_bq = nc.dram_tensor("bq2", [P, 2], F32, kind="ExternalInput")
    d_bk = nc.dram_tensor("bk2", [P, 2], F32, kind="ExternalInput")
    d_vi = nc.dram_tensor("vib1", [1, 256], F32, kind="ExternalInput")
    d_bo = nc.dram_tensor("bo1", [1, D], F32, kind="ExternalInput")
    d_y = nc.dram_tensor("y", [B, QS, D], BF16, kind="ExternalOutput")

    # AV psum packing: chains 0-6 -> bankA, 7-13 -> bankB, 14-15 -> bankC
    AV_SPLIT = [(0, 7), (7, 7), (14, 2)]

    def av_loc(j):
        for t, (base, n) in enumerate(AV_SPLIT):
            if base <= j < base + n:
                return t, j - base
        raise AssertionError

    with tile.TileContext(nc) as tc:
        with (
            tc.tile_pool(name="statics", bufs=1) as st,
            tc.tile_pool(name="dram", bufs=1, space="DRAM") as dram,
        ):
            bq = st.tile([P, 2], F32, tag="bq", name="bq")
            bk = st.tile([P, 2], F32, tag="bk", name="bk")
            lnc = st.tile([P, 1], F32, tag="lnc", name="lnc")
            vib = st.tile([P, 256], F32, tag="vib", name="vib")
            bob = st.tile([P, D], F32, tag="bob", name="bob")
            ident = st.tile([P, P], BF16, tag="ident", name="ident")
            nc.vector.memset(lnc[:], LNC)
            masks.make_identity(nc, ident[:])

            # head pair tiles: partition = (h%2)*64 + hd  (PE base 0/64)
            qTp = [st.tile([P, S], BF16, tag=f"qT{m}", name=f"qT{m}")
                   for m in range(2)]
            kTp = [st.tile([P, S], BF16, tag=f"kT{m}", name=f"kT{m}")
                   for m in range(2)]
            vaug = [st.tile([P, GH * VW], BF16, tag=f"va{i}", name=f"va{i}")
                    for i in range(16)]
            pay = [st.tile([HD, S], BF16, tag=f"pay{h}", name=f"pay{h}")
                   for h in range(GH)]
            woT = st.tile([P, KT * D], BF16, tag="woT", name="woT")
            # merged A2A readback: aoE/aoO[:, (bb*4+gp)*256 + q] for even/odd
            # k-tiles (lower 64 partitions = first head of the pair)
            aoE = st.tile([P, 8 * QS], BF16, tag="aoE", name="aoE")
            aoO = st.tile([P, 8 * QS], BF16, tag="aoO", name="aoO")

            # ---- projections ----
            with (
                tc.tile_pool(name="proj", bufs=1) as pr,
                tc.tile_pool(name="pj", bufs=5, space="PSUM") as pj,
                tc.tile_pool(name="pv", bufs=2, space="PSUM") as pvp,
            ):
                wqT = pr.tile([P, KT * 256], BF16, tag="wqT", name="wqT")
                wkT = pr.tile([P, KT * 256], BF16, tag="wkT", name="wkT")
                wvT = pr.tile([P, KT * 256], BF16, tag="wvT", name="wvT")
                xT = [pr.tile([P, S], BF16, tag=f"x{k}", name=f"x{k}")
                      for k in range(KT)]
                vi1 = pr.tile([1, 256], F32, tag="vi1", name="vi1")
                bo1 = pr.tile([1, D], F32, tag="bo1", name="bo1")
                # x owns the SP queue; weights go via the ACT DGE queue in
                # halves interleaved behind x0 so the PE's first chain group
                # starts ~4us in and never starves on x[k] or w[k]
                wq3 = wqT.rearrange("p (k c) -> p k c", k=KT)
                wk3 = wkT.rearrange("p (k c) -> p k c", k=KT)
                dq3 = d_wq.rearrange("(k p) c -> p k c", k=KT)
                dk3 = d_wk.rearrange("(k p) c -> p k c", k=KT)
                nc.scalar.dma_start(wk3[:, 0:4], dk3[:, 0:4])
                nc.scalar.dma_start(wq3[:, 0:4], dq3[:, 0:4])
                for k in range(0, 3):
                    nc.sync.dma_start(xT[k][:], d_x[k * P:(k + 1) * P, :])
                nc.scalar.dma_start(wk3[:, 4:8], dk3[:, 4:8])
                nc.scalar.dma_start(wq3[:, 4:8], dq3[:, 4:8])
                for k in range(3, KT):
                    nc.sync.dma_start(xT[k][:], d_x[k * P:(k + 1) * P, :])
                nc.scalar.dma_start(bq[:], d_bq[:])
                nc.scalar.dma_start(bk[:], d_bk[:])
                for i in range(16):
                    ones = vaug[i].rearrange("p (h w) -> p h w", w=VW)
                    nc.gpsimd.memset(ones[:, :, HD:VW], 1.0)

                # Q (bias-copy on ACT) / K (on DVE); k-major within groups of
                # chains so the PE streams behind the x DMA; pair 0 first
                qk_chains = []
                for m in range(2):
                    for nb in range(4):
                        qk_chains.append(("K", m, nb))
                        qk_chains.append(("Q", m, nb))
                for g0 in range(0, 16, 4):
                    grp = qk_chains[g0:g0 + 4]
                    tiles = [pj.tile([P, 512], F32, tag="pj", name="pj")
                             for _ in grp]
                    for k in range(KT):
                        for (pk, m, nb), ps in zip(grp, tiles):
                            w = wkT if pk == "K" else wqT
                            ws = w[:, k * 256 + m * P: k * 256 + (m + 1) * P]
                            nc.tensor.matmul(
                                ps[:], ws,
                                xT[k][:, nb * 512:(nb + 1) * 512],
                                start=(k == 0), stop=(k == KT - 1))
                    for (pk, m, nb), ps in zip(grp, tiles):
                        if pk == "K":
                            nc.vector.tensor_scalar(
                                kTp[m][:, nb * 512:(nb + 1) * 512], ps[:],
                                bk[:, m:m + 1], None, ADD)
                        else:
                            nc.scalar.activation(
                                qTp[m][:, nb * 512:(nb + 1) * 512], ps[:],
                                IDN, bias=bq[:, m:m + 1], scale=1.0)

                # V weights + biases now; wo last (needed only at out-proj)
                nc.scalar.dma_start(
                    wvT.rearrange("p (k c) -> p k c", k=KT),
                    d_wv.rearrange("(k p) c -> p k c", k=KT))
                nc.scalar.dma_start(vi1[:], d_vi[:])
                nc.scalar.dma_start(bo1[:], d_bo[:])
                nc.gpsimd.partition_broadcast(vib[:], vi1[:])
                nc.gpsimd.partition_broadcast(bob[:], bo1[:])
                nc.scalar.dma_start(
                    woT.rearrange("p (k c) -> p k c", k=KT),
                    d_wo.rearrange("(k p) c -> p k c", k=KT))

                # V: natural layout [kpos, 4 heads x 64] + ones col
                for sb in range(16):
                    pv = pvp.tile([P, 256], F32, tag="pv", name="pv")
                    for k in range(KT):
                        nc.tensor.matmul(
                            pv[:], xT[k][:, sb * P:(sb + 1) * P],
                            wvT[:, k * 256:(k + 1) * 256],
                            start=(k == 0), stop=(k == KT - 1))
                    dst = vaug[sb].rearrange("p (h w) -> p h w", w=VW)
                    nc.vector.tensor_tensor(
                        dst[:, :, 0:HD],
                        pv.rearrange("p (h w) -> p h w", w=HD),
                        vib.rearrange("p (h w) -> p h w", w=HD), ADD)

            # ---- attention ----
            a_ins = [dram.tile([CORES * HD, QS], BF16, name=f"a_in{h}")
                     for h in range(GH)]
            a_outs = [dram.tile([CORES * HD, QS], BF16, name=f"a_out{h}")
                      for h in range(GH)]

            def readback(h):
                # h 0/1 -> aoE lower/upper half; 2/3 -> aoO
                dst = (aoE if h < 2 else aoO).rearrange(
                    "p (s q) -> p s q", s=8)[(h % 2) * HD:(h % 2 + 1) * HD]
                nc.sync.dma_start(
                    dst, a_outs[h].rearrange("(s p) q -> p s q", s=8))

            with (
                tc.tile_pool(name="exp", bufs=1) as exp_pool,
                tc.tile_pool(name="nrm", bufs=2) as nr,
                tc.tile_pool(name="psc", bufs=4, space="PSUM") as psc,
                tc.tile_pool(name="pav", bufs=1, space="PSUM") as pav,
                tc.tile_pool(name="ptp", bufs=1, space="PSUM") as ptp,
            ):
                LAG = 2

                def drain_head(h, attsb, avt):
                    """Normalize (recip of ones col) -> bf16, emitted at head
                    end on the vector engines; one broadcast multiply per
                    AV psum tile."""
                    at3 = attsb.rearrange("p (c w) -> p c w", w=HD)
                    for t, (base, n) in enumerate(AV_SPLIT):
                        r = nr.tile([P, n], F32, tag=f"rs{t}", name=f"rs{t}")
                        den = avt[t].rearrange("p (c w) -> p c w", w=VW)
                        nc.vector.reciprocal(r[:], den[:, :, HD])
                        rb = r.rearrange("p (c o) -> p c o", o=1).broadcast_to(
                            (P, n, HD))
                        nc.vector.tensor_tensor(
                            at3[:, base:base + n], den[:, :, 0:HD], rb, MUL)

                def transpose_group(h, attsb, g):
                    """PE transpose of 8 chains + copy into pay + A2A dma of
                    the half."""
                    tp = ptp.tile([HD, 8 * P], BF16, tag="tp", name="tp")
                    for j8 in range(8):
                        j = g * 8 + j8
                        nc.tensor.matmul(
                            tp[:, j8 * P:(j8 + 1) * P],
                            attsb[:, j * HD:(j + 1) * HD], ident[:],
                            is_transpose=True, start=(j8 == 0),
                            stop=(j8 == 7), skip_group_check=True)
                    if g == 0:
                        nc.scalar.copy(pay[h][:, 0:1024], tp[:])
                    else:
                        nc.vector.tensor_copy(pay[h][:, 1024:2048], tp[:])
                    dst = a_ins[h].rearrange("(j r) q -> r j q", j=CORES)
                    src = pay[h].rearrange("p (j q) -> p j q", j=CORES)
                    nc.sync.dma_start(dst[:, g * 4:(g + 1) * 4],
                                      src[:, g * 4:(g + 1) * 4])

                def issue_cc(h):
                    nc.gpsimd.collective_compute(
                        "AllToAll",
                        mybir.AluOpType.bypass,
                        replica_groups=[list(range(CORES))],
                        ins=[a_ins[h][:]],
                        outs=[a_outs[h][:]],
                    )

                prev = None  # deferred drain state of head h-1
                for h in range(GH):
                    ksl = kTp[h // 2][(h % 2) * HD:(h % 2 + 1) * HD]
                    qsl = qTp[h // 2][(h % 2) * HD:(h % 2 + 1) * HD]
                    # one tile per (kb, qt) exp instruction: single writer,
                    # so cross-engine exps of one key block never serialize
                    ex = [[exp_pool.tile([P, 512], BF16, tag=f"ex{i}_{q}",
                                         name=f"ex{i}_{q}") for q in range(4)]
                          for i in range(16)]
                    avt = [pav.tile([P, n * VW], F32, tag=f"av{t}",
                                    name=f"av{t}")
                           for t, (_, n) in enumerate(AV_SPLIT)]

                    def av_half(kb, half, ex=ex, avt=avt, h=h):
                        # 8 chains: stationary = exp tile q-slice (M=128),
                        # moving = V+ones (N=65); accumulate over key blocks
                        for j in range(half * 8, half * 8 + 8):
                            t, jj = av_loc(j)
                            nc.tensor.matmul(
                                avt[t][:, jj * VW:(jj + 1) * VW],
                                ex[kb][j // 4][:, (j % 4) * P:
                                               (j % 4 + 1) * P],
                                vaug[kb][:, h * VW:(h + 1) * VW],
                                start=(kb == 0 and jj == 0),
                                stop=(kb == 15 and jj == AV_SPLIT[t][1] - 1),
                                skip_group_check=True)

                    for kb in range(16):
                        for qt in range(4):
                            # trailing-AV halves interleaved before the score
                            # pairs for uniform PE production pacing
                            if kb >= LAG and qt == 0:
                                av_half(kb - LAG, 0)
                            elif kb >= LAG and qt == 2:
                                av_half(kb - LAG, 1)
                            sc = psc.tile([P, 512], F32, tag="sc", name="sc")
                            qo = qt * 512
                            nc.tensor.matmul(
                                sc[:], ksl[:, kb * P:(kb + 1) * P],
                                qsl[:, qo:qo + 512],
                                start=True, stop=True)
                            dst = ex[kb][qt][:]
                            pick = PAT[kb * 4 + qt]
                            if pick == "A":
                                nc.scalar.activation(
                                    dst, sc[:], EXP,
                                    bias=lnc[:, 0:1], scale=1.0)
                            else:
                                nc.vector.tensor_scalar(
                                    dst.bitcast(U16), sc[:],
                                    A16, B16, MUL, ADD)
                        # deferred drain of the previous head, spread so the
                        # PE/Pool FIFOs never block at the boundary
                        if prev is not None:
                            ph, pattsb = prev
                            if kb == 0:
                                transpose_group(ph, pattsb, 0)
                            elif kb == 1:
                                transpose_group(ph, pattsb, 1)
                            elif kb == 2:
                                issue_cc(ph)
                                if ph >= 1:
                                    readback(ph - 1)
                    for kb in range(16 - LAG, 16):
                        av_half(kb, 0)
                        av_half(kb, 1)

                    attsb = nr.tile([P, 16 * HD], BF16, tag="attsb",
                                    name="attsb")
                    drain_head(h, attsb, avt)
                    prev = (h, attsb)

                # tail: drain head 3 immediately
                transpose_group(3, prev[1], 0)
                transpose_group(3, prev[1], 1)
                issue_cc(3)
                readback(2)
                readback(3)

            # ---- out projection (my 256-row slice of each batch) ----
            with (
                tc.tile_pool(name="po", bufs=1, space="PSUM") as po,
                tc.tile_pool(name="yo", bufs=4) as yo,
            ):
                tiles = {}
                for key in [(bb, m, n) for bb in reversed(range(B))
                            for m in reversed(range(2))
                            for n in reversed(range(2))]:
                    tiles[key] = po.tile([P, 512], F32,
                                         tag="po{}{}{}".format(*key),
                                         name="po{}{}{}".format(*key))
                chains = [(bb, m, n, tiles[(bb, m, n)])
                          for bb in range(B) for m in range(2)
                          for n in range(2)]
                # three waves: even k-tiles full-K (aoE, after c0+c1), then
                # the head-2 and head-3 K=64 halves of the odd k-tiles
                # (after c2 / c3) so the PE keeps running while c3 lands
                for phase in range(3):
                    for bb, m, n, ps in chains:
                        for ki in range(4):
                            k = ki * 2 + (1 if phase > 0 else 0)
                            c0 = (bb * 4 + ki) * QS + m * P
                            if phase == 0:
                                src = aoE[:, c0:c0 + P]
                                wos = woT[:, k * D + n * 512:
                                          k * D + (n + 1) * 512]
                            else:
                                src = aoO[(phase - 1) * HD:phase * HD,
                                          c0:c0 + P]
                                wos = woT.rearrange(
                                    "p (k c) -> p k c", k=KT)[
                                    (phase - 1) * HD:phase * HD, k,
                                    n * 512:(n + 1) * 512]
                            nc.tensor.matmul(
                                ps[:], src, wos,
                                start=(phase == 0 and ki == 0),
                                stop=(phase == 2 and ki == 3))
                for ci, (bb, m, n, ps) in enumerate(chains):
                    ys = yo.tile([P, 512], BF16, tag="ys", name="ys")
                    nc.vector.tensor_tensor(
                        ys[:], ps[:], bob[:, n * 512:(n + 1) * 512], ADD)
                    nc.sync.dma_start(
                        d_y[bb, m * P:(m + 1) * P, n * 512:(n + 1) * 512],
                        ys[:])

    nc.compile()
    return nc


def get_nc():
    if "nc" not in _CACHE:
        _CACHE["nc"] = _build_nc()
    return _CACHE["nc"]


def make_in_maps(x, Wq, bq, Wk, bk, Wv, bv, Wo, bo):
    bf16 = ml_dtypes.bfloat16
    x = np.asarray(x, dtype=np.float32)
    Wq, Wk, Wv, Wo = (np.asarray(w, dtype=np.float32) for w in (Wq, Wk, Wv, Wo))
    bq, bk, bv, bo = (np.asarray(v, dtype=np.float32) for v in (bq, bk, bv, bo))
    scale = 1.0 / np.sqrt(np.float32(HD))

    wo16 = np.ascontiguousarray(Wo.T).astype(bf16)
    bo1 = bo.reshape(1, D)

    in_maps = []
    for cc in range(CORES):
        b, g = cc // 4, cc % 4
        sl = slice(g * 256, (g + 1) * 256)
        x16 = np.ascontiguousarray(x[b].T).astype(bf16)
        wq16 = np.ascontiguousarray((Wq[sl, :] * scale).T).astype(bf16)
        wk16 = np.ascontiguousarray(Wk[sl, :].T).astype(bf16)
        wv16 = np.ascontiguousarray(Wv[sl, :].T).astype(bf16)
        pp = np.arange(P)
        bq2 = np.stack([bq[g * 256 + m * P + pp] * scale for m in range(2)],
                       axis=1).astype(np.float32)
        bk2 = np.stack([bk[g * 256 + m * P + pp] for m in range(2)],
                       axis=1).astype(np.float32)
        vib1 = bv[sl].reshape(1, 256).astype(np.float32)
        in_maps.append({
            "x16": x16, "wq16": wq16, "wk16": wk16, "wv16": wv16,
            "wo16": wo16, "bq2": np.ascontiguousarray(bq2),
            "bk2": np.ascontiguousarray(bk2), "vib1": vib1, "bo1": bo1,
        })
    return in_maps


def assemble(results):
    out = np.empty((B, S, D), dtype=np.float32)
    for c in range(CORES):
        out[:, c * QS:(c + 1) * QS, :] = np.asarray(
            results[c]["y"], dtype=np.float32)
    return out


def kernel(**inputs):
    from concourse.bass_utils import run_bass_kernel_spmd

    nc = get_nc()
    in_maps = make_in_maps(**inputs)
    res = run_bass_kernel_spmd(nc, in_maps, list(range(CORES)), trace=False)
    return assemble(res.results)


# revision 25
# speedup vs baseline: 1.0945x; 1.0892x over previous
"""MHA (B=2, S=2048, D=1024, H=16) on 8 Trainium2 NeuronCores - bf16.

Sharding: core c = (batch b = c//4, head-group g = c%4, 4 heads each).

v3 structural changes over the 223us baseline (cost model: matmul time =
moving-free-size cycles, independent of contraction/partition count):
 - AV matmul flipped: exp-scores tile [128k,128q] is the STATIONARY
   operand, V(+ones) [128k,65] the moving one: AV drops from 32768 to
   16640 PE cycles per head. Output lands as [q, 64hd+denom] in PSUM,
   is normalized by the per-partition reciprocal of the ones column
   (tensor_scalar) and PE-transposed (identity matmuls) back to [hd, q].
 - softmax exp on THREE engines (ACT exact / DVE+Pool schraudolph),
   strict A,D,A,P pattern; scores PSUM 4 banks deep and AV trailing 5
   key-blocks so the PE never waits on the exp round-trip.
 - per-head drain (transpose/copy/A2A-dma/collective) is emitted INSIDE
   the next head's kb loop so neither the PE FIFO nor Pool.SEQ stalls at
   head boundaries.
 - four per-head AllToAlls (15us fixed cost each, serialized on the
   collective cores): c0 starts one head earlier than the baseline's
   merged head-pair collective and c3 lands ~6us sooner.
 - out-projection in two full-K waves (even k-tiles, then odd).
 - weight DMAs merged (one copy per W) on the ACT DGE queue; x tiles
   keep the whole SP queue so the PE starts earlier and never starves.
"""

import numpy as np
import ml_dtypes

B, S, D, H = 2, 2048, 1024, 16
HD = D // H          # 64
GH = 4               # heads per core
CORES = 8
QS = S // CORES      # 256 per-core final sequence slice (per batch)
P = 128
KT = D // P          # 8 k-tiles of the model dim
VW = HD + 1          # 65: per-head V width incl. ones column

A16 = 128.0 / float(np.log(2.0))   # schraudolph slope (bf16-bit domain)
B16 = 16255.35                     # schraudolph offset (tuned)
LNC = 0.03638                      # ln(c): ACT path matches schraudolph scale

_CACHE = {}

# exp engine shares per head (64 tiles): ACT exact / DVE / Pool schraudolph
EXP_SHARES = {"A": 33, "D": 31}


def _exp_engine_pattern():
    shares = dict(EXP_SHARES)
    counts = {k: 0 for k in shares}
    seq = []
    for i in range(64):
        pick = max(shares, key=lambda k: shares[k] * (i + 1) / 64 - counts[k])
        counts[pick] += 1
        seq.append(pick)
    return seq


PAT = _exp_engine_pattern()
# force the last key-block's final tiles onto ACT: DVE then finishes its
# exp share early and can run the drain normalize the moment the AV
# chains stop, shortening the pay3 -> collective-3 critical path
for _i in (62, 63):
    if PAT[_i] == "D":
        _j = max(k for k in range(60) if PAT[k] == "A")
        PAT[_i], PAT[_j] = PAT[_j], PAT[_i]


def _build_nc():
    import concourse.mybir as mybir
    import concourse.tile as tile
    from concourse import bacc
    from concourse import masks

    F32 = mybir.dt.float32
    BF16 = mybir.dt.bfloat16
    U16 = mybir.dt.uint16
    EXP = mybir.ActivationFunctionType.Exp
    IDN = mybir.ActivationFunctionType.Identity
    MUL = mybir.AluOpType.mult
    ADD = mybir.AluOpType.add

    nc = bacc.Bacc("TRN2", target_bir_lowering=False, debug=False,
                   num_devices=CORES)

    d_x = nc.dram_tensor("x16", [D, S], BF16, kind="ExternalInput")
    d_wq = nc.dram_tensor("wq16", [D, 256], BF16, kind="ExternalInput")
    d_wk = nc.dram_tensor("wk16", [D, 256], BF16, kind="ExternalInput")
    d_wv = nc.dram_tensor("wv16", [D, 256], BF16, kind="ExternalInput")
    d_wo = nc.dram_tensor("wo16", [D, D], BF16, kind="ExternalInput")
    d_bq = nc.dram_tensor("bq2", [P, 2], F32, kind="ExternalInput")
    d_bk = nc.dram_tensor("bk2", [P, 2], F32, kind="ExternalInput")
    d_vi = nc.dram_tensor("vib1", [1, 256], F32, kind="ExternalInput")
    d_bo = nc.dram_tensor("bo1", [1, D], F32, kind="ExternalInput")
    d_y = nc.dram_tensor("y", [B, QS, D], BF16, kind="ExternalOutput")

    # AV psum packing: chains 0-6 -> bankA, 7-13 -> bankB, 14-15 -> bankC
    AV_SPLIT = [(0, 7), (7, 7), (14, 2)]

    def av_loc(j):
        for t, (base, n) in enumerate(AV_SPLIT):
            if base <= j < base + n:
                return t, j - base
        raise AssertionError

    with tile.TileContext(nc) as tc:
        with (
            tc.tile_pool(name="statics", bufs=1) as st,
            tc.tile_pool(name="dram", bufs=1, space="DRAM") as dram,
        ):
            bq = st.tile([P, 2], F32, tag="bq", name="bq")
            bk = st.tile([P, 2], F32, tag="bk", name="bk")
            lnc = st.tile([P, 1], F32, tag="lnc", name="lnc")
            vib = st.tile([P, 256], F32, tag="vib", name="vib")
            bob = st.tile([P, D], F32, tag="bob", name="bob")
            ident = st.tile([P, P], BF16, tag="ident", name="ident")
            nc.vector.memset(lnc[:], LNC)
            masks.make_identity(nc, ident[:])

            # head pair tiles: partition = (h%2)*64 + hd  (PE base 0/64)
            qTp = [st.tile([P, S], BF16, tag=f"qT{m}", name=f"qT{m}")
                   for m in range(2)]
            kTp = [st.tile([P, S], BF16, tag=f"kT{m}", name=f"kT{m}")
                   for m in range(2)]
            vaug = [st.tile([P, GH * VW], BF16, tag=f"va{i}", name=f"va{i}")
                    for i in range(16)]
            pay = [st.tile([HD, S], BF16, tag=f"pay{h}", name=f"pay{h}")
                   for h in range(GH)]
            woT = st.tile([P, KT * D], BF16, tag="woT", name="woT")
            # merged A2A readback: aoE/aoO[:, (bb*4+gp)*256 + q] for even/odd
            # k-tiles (lower 64 partitions = first head of the pair)
            aoE = st.tile([P, 8 * QS], BF16, tag="aoE", name="aoE")
            aoO = st.tile([P, 8 * QS], BF16, tag="aoO", name="aoO")

            # ---- projections ----
            with tc.tile_pool(name="proj", bufs=1) as pr:
                wqT = pr.tile([P, KT * 256], BF16, tag="wqT", name="wqT")
                wkT = pr.tile([P, KT * 256], BF16, tag="wkT", name="wkT")
                wvT = pr.tile([P, KT * 256], BF16, tag="wvT", name="wvT")
                xT = [pr.tile([P, S], BF16, tag=f"x{k}", name=f"x{k}")
                      for k in range(KT)]
                vi1 = pr.tile([1, 256], F32, tag="vi1", name="vi1")
                bo1 = pr.tile([1, D], F32, tag="bo1", name="bo1")
                # x owns the SP queue; weights go via the ACT DGE queue in
                # halves interleaved behind x0 so the PE's first chain group
                # starts ~4us in and never starves on x[k] or w[k]
                wq3 = wqT.rearrange("p (k c) -> p k c", k=KT)
                wk3 = wkT.rearrange("p (k c) -> p k c", k=KT)
                dq3 = d_wq.rearrange("(k p) c -> p k c", k=KT)
                dk3 = d_wk.rearrange("(k p) c -> p k c", k=KT)
                nc.scalar.dma_start(wk3[:, 0:4], dk3[:, 0:4])
                nc.scalar.dma_start(wq3[:, 0:4], dq3[:, 0:4])
                for k in range(0, 4):
                    nc.sync.dma_start(xT[k][:], d_x[k * P:(k + 1) * P, :])
                nc.scalar.dma_start(wk3[:, 4:8], dk3[:, 4:8])
                nc.scalar.dma_start(wq3[:, 4:8], dq3[:, 4:8])
                for k in range(4, KT):
                    nc.sync.dma_start(xT[k][:], d_x[k * P:(k + 1) * P, :])
                nc.scalar.dma_start(bq[:], d_bq[:])
                nc.scalar.dma_start(bk[:], d_bk[:])
                for i in range(16):
                    ones = vaug[i].rearrange("p (h w) -> p h w", w=VW)
                    nc.gpsimd.memset(ones[:, :, HD:VW], 1.0)

                # Q (bias-copy on ACT) / K (on DVE); k-major within groups
                # of EIGHT chains (all 8 PSUM banks) so each k-step costs
                # 8x512 PE cycles and the x DMA feed stays ahead; pair 0
                # first
                qk_chains = []
                for m in range(2):
                    for nb in range(4):
                        qk_chains.append(("K", m, nb))
                        qk_chains.append(("Q", m, nb))
                with tc.tile_pool(name="pj", bufs=8, space="PSUM") as pj:
                    for g0 in range(0, 16, 8):
                        grp = qk_chains[g0:g0 + 8]
                        tiles = [pj.tile([P, 512], F32, tag="pj", name="pj")
                                 for _ in grp]
                        for k in range(KT):
                            for (pk, m, nb), ps in zip(grp, tiles):
                                w = wkT if pk == "K" else wqT
                                ws = w[:, k * 256 + m * P:
                                       k * 256 + (m + 1) * P]
                                nc.tensor.matmul(
                                    ps[:], ws,
                                    xT[k][:, nb * 512:(nb + 1) * 512],
                                    start=(k == 0), stop=(k == KT - 1))
                        for (pk, m, nb), ps in zip(grp, tiles):
                            if pk == "K":
                                nc.vector.tensor_scalar(
                                    kTp[m][:, nb * 512:(nb + 1) * 512],
                                    ps[:], bk[:, m:m + 1], None, ADD)
                            else:
                                nc.scalar.activation(
                                    qTp[m][:, nb * 512:(nb + 1) * 512],
                                    ps[:], IDN, bias=bq[:, m:m + 1],
                                    scale=1.0)

                # V weights + biases now; wo last (needed only at out-proj)
                nc.scalar.dma_start(
                    wvT.rearrange("p (k c) -> p k c", k=KT),
                    d_wv.rearrange("(k p) c -> p k c", k=KT))
                nc.scalar.dma_start(vi1[:], d_vi[:])
                nc.scalar.dma_start(bo1[:], d_bo[:])
                nc.gpsimd.partition_broadcast(vib[:], vi1[:])
                nc.gpsimd.partition_broadcast(bob[:], bo1[:])
                nc.scalar.dma_start(
                    woT.rearrange("p (k c) -> p k c", k=KT),
                    d_wo.rearrange("(k p) c -> p k c", k=KT))

                # V: natural layout [kpos, 4 heads x 64] + ones col
                with tc.tile_pool(name="pv", bufs=4, space="PSUM") as pvp:
                    for sb in range(16):
                        pv = pvp.tile([P, 256], F32, tag="pv", name="pv")
                        for k in range(KT):
                            nc.tensor.matmul(
                                pv[:], xT[k][:, sb * P:(sb + 1) * P],
                                wvT[:, k * 256:(k + 1) * 256],
                                start=(k == 0), stop=(k == KT - 1))
                        dst = vaug[sb].rearrange("p (h w) -> p h w", w=VW)
                        nc.vector.tensor_tensor(
                            dst[:, :, 0:HD],
                            pv.rearrange("p (h w) -> p h w", w=HD),
                            vib.rearrange("p (h w) -> p h w", w=HD), ADD)

            # ---- attention ----
            a_ins = [dram.tile([CORES * HD, QS], BF16, name=f"a_in{h}")
                     for h in range(GH)]
            a_outs = [dram.tile([CORES * HD, QS], BF16, name=f"a_out{h}")
                      for h in range(GH)]

            def readback(h):
                # h 0/1 -> aoE lower/upper half; 2/3 -> aoO; two DMAs so the
                # out-proj waves can start on the first batch's blocks
                dst = (aoE if h < 2 else aoO).rearrange(
                    "p (s q) -> p s q", s=8)[(h % 2) * HD:(h % 2 + 1) * HD]
                src_ = a_outs[h].rearrange("(s p) q -> p s q", s=8)
                nc.sync.dma_start(dst[:, 0:4], src_[:, 0:4])
                nc.sync.dma_start(dst[:, 4:8], src_[:, 4:8])

            with (
                tc.tile_pool(name="exp", bufs=1) as exp_pool,
                tc.tile_pool(name="nrm", bufs=2) as nr,
                tc.tile_pool(name="psc", bufs=4, space="PSUM") as psc,
                tc.tile_pool(name="pav", bufs=1, space="PSUM") as pav,
                tc.tile_pool(name="ptp", bufs=1, space="PSUM") as ptp,
            ):
                LAG = 2

                def drain_head(h, attsb, avt):
                    """Normalize (recip of ones col) -> bf16, emitted at head
                    end on the vector engines; one broadcast multiply per
                    AV psum tile."""
                    at3 = attsb.rearrange("p (c w) -> p c w", w=HD)
                    for t, (base, n) in enumerate(AV_SPLIT):
                        r = nr.tile([P, n], F32, tag=f"rs{t}", name=f"rs{t}")
                        den = avt[t].rearrange("p (c w) -> p c w", w=VW)
                        nc.vector.reciprocal(r[:], den[:, :, HD])
                        rb = r.rearrange("p (c o) -> p c o", o=1).broadcast_to(
                            (P, n, HD))
                        nc.vector.tensor_tensor(
                            at3[:, base:base + n], den[:, :, 0:HD], rb, MUL)

                def transpose_group(h, attsb, g):
                    """PE transpose of 8 chains + copy into pay + A2A dma of
                    the half."""
                    tp = ptp.tile([HD, 8 * P], BF16, tag="tp", name="tp")
                    for j8 in range(8):
                        j = g * 8 + j8
                        nc.tensor.matmul(
                            tp[:, j8 * P:(j8 + 1) * P],
                            attsb[:, j * HD:(j + 1) * HD], ident[:],
                            is_transpose=True, start=(j8 == 0),
                            stop=(j8 == 7), skip_group_check=True)
                    if g == 0:
                        nc.scalar.copy(pay[h][:, 0:1024], tp[:])
                    else:
                        nc.vector.tensor_copy(pay[h][:, 1024:2048], tp[:])
                        dst = a_ins[h].rearrange("(j r) q -> r j q", j=CORES)
                        src = pay[h].rearrange("p (j q) -> p j q", j=CORES)
                        nc.sync.dma_start(dst, src)

                def issue_cc(h):
                    nc.gpsimd.collective_compute(
                        "AllToAll",
                        mybir.AluOpType.bypass,
                        replica_groups=[list(range(CORES))],
                        ins=[a_ins[h][:]],
                        outs=[a_outs[h][:]],
                    )

                prev = None  # deferred drain state of head h-1
                for h in range(GH):
                    ksl = kTp[h // 2][(h % 2) * HD:(h % 2 + 1) * HD]
                    qsl = qTp[h // 2][(h % 2) * HD:(h % 2 + 1) * HD]
                    # one tile per (kb, qt) exp instruction: single writer,
                    # so cross-engine exps of one key block never serialize
                    ex = [[exp_pool.tile([P, 512], BF16, tag=f"ex{i}_{q}",
                                         name=f"ex{i}_{q}") for q in range(4)]
                          for i in range(16)]
                    avt = [pav.tile([P, n * VW], F32, tag=f"av{t}",
                                    name=f"av{t}")
                           for t, (_, n) in enumerate(AV_SPLIT)]

                    def av_half(kb, half, ex=ex, avt=avt, h=h):
                        # 8 chains: stationary = exp tile q-slice (M=128),
                        # moving = V+ones (N=65); accumulate over key blocks
                        for j in range(half * 8, half * 8 + 8):
                            t, jj = av_loc(j)
                            nc.tensor.matmul(
                                avt[t][:, jj * VW:(jj + 1) * VW],
                                ex[kb][j // 4][:, (j % 4) * P:
                                               (j % 4 + 1) * P],
                                vaug[kb][:, h * VW:(h + 1) * VW],
                                start=(kb == 0 and jj == 0),
                                stop=(kb == 15 and jj == AV_SPLIT[t][1] - 1),
                                skip_group_check=True)

                    for kb in range(16):
                        for qt in range(4):
                            # trailing-AV halves interleaved before the score
                            # pairs for uniform PE production pacing
                            if kb >= LAG and qt == 0:
                                av_half(kb - LAG, 0)
                            elif kb >= LAG and qt == 2:
                                av_half(kb - LAG, 1)
                            sc = psc.tile([P, 512], F32, tag="sc", name="sc")
                            qo = qt * 512
                            nc.tensor.matmul(
                                sc[:], ksl[:, kb * P:(kb + 1) * P],
                                qsl[:, qo:qo + 512],
                                start=True, stop=True)
                            dst = ex[kb][qt][:]
                            pick = PAT[kb * 4 + qt]
                            if pick == "A":
                                nc.scalar.activation(
                                    dst, sc[:], EXP,
                                    bias=lnc[:, 0:1], scale=1.0)
                            else:
                                nc.vector.tensor_scalar(
                                    dst.bitcast(U16), sc[:],
                                    A16, B16, MUL, ADD)
                        # deferred drain of the previous head, spread so the
                        # PE/Pool FIFOs never block at the boundary
                        if prev is not None:
                            ph, pattsb = prev
                            if kb == 0:
                                transpose_group(ph, pattsb, 0)
                            elif kb == 1:
                                transpose_group(ph, pattsb, 1)
                            elif kb == 2:
                                issue_cc(ph)
                                if ph >= 1:
                                    readback(ph - 1)
                    for kb in range(16 - LAG, 16):
                        av_half(kb, 0)
                        av_half(kb, 1)

                    attsb = nr.tile([P, 16 * HD], BF16, tag="attsb",
                                    name="attsb")
                    drain_head(h, attsb, avt)
                    prev = (h, attsb)

                # tail: drain head 3 immediately
                transpose_group(3, prev[1], 0)
                transpose_group(3, prev[1], 1)
                issue_cc(3)
                readback(2)
                readback(3)

            # ---- out projection (my 256-row slice of each batch) ----
            with (
                tc.tile_pool(name="po", bufs=1, space="PSUM") as po,
                tc.tile_pool(name="yo", bufs=4) as yo,
            ):
                tiles = {}
                for key in [(bb, m, n) for bb in reversed(range(B))
                            for m in reversed(range(2))
                            for n in reversed(range(2))]:
                    tiles[key] = po.tile([P, 512], F32,
                                         tag="po{}{}{}".format(*key),
                                         name="po{}{}{}".format(*key))
                chains = [(bb, m, n, tiles[(bb, m, n)])
                          for bb in range(B) for m in range(2)
                          for n in range(2)]
                # warm-up filler: the PE would otherwise idle ~20us waiting
                # for the last collective and the final wave would dispatch
                # at the cold p-state clock (~2.6x slower). Throwaway
                # matmuls into chain0's bank keep the clock hot; each
                # start=True re-marks the bank pending-zero, and the real
                # even-wave start below re-zeroes it again, so no garbage
                # survives. Reads aoE which is ready two collectives early.
                warm = tiles[(0, 0, 0)]
                for _ in range(85):
                    nc.tensor.matmul(
                        warm[:], aoE[:, 0:P], woT[:, 0:512],
                        start=True, stop=True, skip_group_check=True)
                # three waves: even k-tiles full-K (aoE, after c0+c1), then
                # the head-2 and head-3 K=64 halves of the odd k-tiles
                # (after c2 / c3) so the PE keeps running while c3 lands
                for phase in range(3):
                    for bb, m, n, ps in chains:
                        for ki in range(4):
                            k = ki * 2 + (1 if phase > 0 else 0)
                            c0 = (bb * 4 + ki) * QS + m * P
                            if phase == 0:
                                src = aoE[:, c0:c0 + P]
                                wos = woT[:, k * D + n * 512:
                                          k * D + (n + 1) * 512]
                            else:
                                src = aoO[(phase - 1) * HD:phase * HD,
                                          c0:c0 + P]
                                wos = woT.rearrange(
                                    "p (k c) -> p k c", k=KT)[
                                    (phase - 1) * HD:phase * HD, k,
                                    n * 512:(n + 1) * 512]
                            nc.tensor.matmul(
                                ps[:], src, wos,
                                start=(phase == 0 and ki == 0),
                                stop=(phase == 2 and ki == 3))
                for ci, (bb, m, n, ps) in enumerate(chains):
                    ys = yo.tile([P, 512], BF16, tag="ys", name="ys")
                    nc.vector.tensor_tensor(
                        ys[:], ps[:], bob[:, n * 512:(n + 1) * 512], ADD)
                    nc.sync.dma_start(
                        d_y[bb, m * P:(m + 1) * P, n * 512:(n + 1) * 512],
                        ys[:])

    nc.compile()
    return nc


def get_nc():
    if "nc" not in _CACHE:
        _CACHE["nc"] = _build_nc()
    return _CACHE["nc"]


def make_in_maps(x, Wq, bq, Wk, bk, Wv, bv, Wo, bo):
    bf16 = ml_dtypes.bfloat16
    x = np.asarray(x, dtype=np.float32)
    Wq, Wk, Wv, Wo = (np.asarray(w, dtype=np.float32) for w in (Wq, Wk, Wv, Wo))
    bq, bk, bv, bo = (np.asarray(v, dtype=np.float32) for v in (bq, bk, bv, bo))
    scale = 1.0 / np.sqrt(np.float32(HD))

    wo16 = np.ascontiguousarray(Wo.T).astype(bf16)
    bo1 = bo.reshape(1, D)

    in_maps = []
    for cc in range(CORES):
        b, g = cc // 4, cc % 4
        sl = slice(g * 256, (g + 1) * 256)
        x16 = np.ascontiguousarray(x[b].T).astype(bf16)
        wq16 = np.ascontiguousarray((Wq[sl, :] * scale).T).astype(bf16)
        wk16 = np.ascontiguousarray(Wk[sl, :].T).astype(bf16)
        wv16 = np.ascontiguousarray(Wv[sl, :].T).astype(bf16)
        pp = np.arange(P)
        bq2 = np.stack([bq[g * 256 + m * P + pp] * scale for m in range(2)],
                       axis=1).astype(np.float32)
        bk2 = np.stack([bk[g * 256 + m * P + pp] for m in range(2)],
                       axis=1).astype(np.float32)
        vib1 = bv[sl].reshape(1, 256).astype(np.float32)
        in_maps.append({
            "x16": x16, "wq16": wq16, "wk16": wk16, "wv16": wv16,
            "wo16": wo16, "bq2": np.ascontiguousarray(bq2),
            "bk2": np.ascontiguousarray(bk2), "vib1": vib1, "bo1": bo1,
        })
    return in_maps


def assemble(results):
    out = np.empty((B, S, D), dtype=np.float32)
    for c in range(CORES):
        out[:, c * QS:(c + 1) * QS, :] = np.asarray(
            results[c]["y"], dtype=np.float32)
    return out


def kernel(**inputs):
    from concourse.bass_utils import run_bass_kernel_spmd

    nc = get_nc()
    in_maps = make_in_maps(**inputs)
    res = run_bass_kernel_spmd(nc, in_maps, list(range(CORES)), trace=False)
    return assemble(res.results)
